# revision 1
# baseline (speedup 1.0000x reference)
"""Chebyshev GCN (10 layers, K=3, N=50000, E=1.2M) on 8 Trainium2 NeuronCores.

Node-sharded graph-parallel design:
  - Nodes are permuted (degree-sorted, dealt round-robin over cores) and
    sharded 8 ways; each core owns NPC=6272 row slots (6250 real + 22 dummy).
  - SpMM uses the separable Chebyshev weight w = -dinv[row]*dinv[col]:
        u = dinv * h              (node-major, uploaded + AllGathered)
        S[r] = sum_d u[colpad[r,d]]   (MoE dma_gather + DVE strided reduce)
        Lhat(h) = -dinv * S
  - Gather indices are int16, so the 50176-row u-buffer is split in two
    25088-row halves (cores 0-3 / 4-7), one gather call per (tile, half);
    padding slots point at an always-zero dummy row.
  - Dense compute is feature-major ([64, nodes]) on the PE; BatchNorm stats
    use a tiny AllReduce. Layer 1 (IN_F=128) uses Lhat(x)@W = Lhat(x@W) so
    every graph op runs at width 64.
"""
import os
import sys
sys.path.insert(0, "/opt/trn_rl_repo")

STAGE = 99
NMID = 9

import numpy as np
import concourse.bass as bass
import concourse.bacc as bacc
import concourse.mybir as mybir
import concourse.tile as tile
from concourse import masks, library_config
from concourse.bass_utils import run_bass_kernel_spmd

# ---------------- problem constants (hardcoded per spec) ----------------
N = 50000
E = 1_200_000
IN_F = 128
C = 64
OUT_F = 2
EPS = 1e-5
NCORES = 8
NTILES = 49
NPC = NTILES * 128            # 6272 row slots per core
REAL_PC = 6250                # real rows per core
NTOT = NCORES * NPC           # 50176
HALF = NTOT // 2              # 25088
ZROW = HALF - 1               # zero row (dummy) valid for both halves
LAST_REAL = REAL_PC - 48 * 128  # 106 real rows in tile 48

FP = mybir.dt.float32
I16 = mybir.dt.int16
AF = mybir.ActivationFunctionType
OP = mybir.AluOpType


# ======================= host-side preprocessing =======================

def preprocess(x, edge_index):
    x = np.asarray(x, dtype=np.float32)
    ei = np.asarray(edge_index)
    row = ei[0].astype(np.int64)
    col = ei[1].astype(np.int64)

    deg = np.bincount(row, minlength=N).astype(np.float32)
    dinv = np.where(deg > 0, 1.0 / np.sqrt(np.maximum(deg, 1.0)), 0.0).astype(np.float32)

    # pin half membership by degree parity-dealing, then stratify exactly
    order0 = np.argsort(-deg, kind="stable")
    halves = [order0[0::2], order0[1::2]]          # 25000 nodes each
    memberA = np.zeros(N, bool)
    memberA[halves[0]] = True
    isA_edge = memberA[col]
    cntA = np.bincount(row[isA_edge], minlength=N).astype(np.int64)
    core_of = np.empty(N, np.int64)
    local_of = np.empty(N, np.int64)
    nodes_by_core = [None] * NCORES
    for h, nodes in enumerate(halves):
        dn = deg[nodes].astype(np.int64)
        serp = np.where(dn % 2 == 0, cntA[nodes], -cntA[nodes])
        oh = nodes[np.lexsort((serp, -dn))]
        core_of[oh] = 4 * h + np.arange(len(oh)) % 4
        local_of[oh] = np.arange(len(oh)) // 4
        for c in range(4):
            nodes_by_core[4 * h + c] = oh[c::4]
    bufpos = core_of * NPC + local_of

    # per-edge slot computation
    e_rb = bufpos[row]
    e_cb = bufpos[col]
    e_isA = e_cb < HALF
    key = e_rb * 2 + (~e_isA)
    eorder = np.lexsort((e_cb, key))
    ks = key[eorder]
    grp_start = np.r_[0, np.flatnonzero(np.diff(ks)) + 1]
    starts = np.zeros(len(ks), np.int64)
    starts[grp_start] = grp_start
    np.maximum.accumulate(starts, out=starts)
    rank = np.arange(len(ks)) - starts
    e_rank = np.empty(E, np.int64)
    e_rank[eorder] = rank

    cntB = deg.astype(np.int64) - cntA
    lA = np.zeros(NTOT, np.int64)
    lB = np.zeros(NTOT, np.int64)
    lA[bufpos] = cntA
    lB[bufpos] = cntB
    D_A = np.maximum(lA.reshape(NCORES, NTILES, 128).max(axis=(0, 2)), 1)
    D_B = np.maximum(lB.reshape(NCORES, NTILES, 128).max(axis=(0, 2)), 1)

    offs_A = np.zeros(NTILES, np.int64)
    offs_B = np.zeros(NTILES, np.int64)
    cur = 0
    for t in range(NTILES):
        offs_A[t] = cur
        cur += 8 * int(D_A[t])
        offs_B[t] = cur
        cur += 8 * int(D_B[t])
    totcols = int(cur)

    idx16 = np.full((NCORES, 16, totcols), ZROW, np.int16)
    e_core = e_rb // NPC
    e_loc = e_rb % NPC
    e_t = e_loc // 128
    e_r = e_loc % 128
    e_val = np.where(e_isA, e_cb, e_cb - HALF).astype(np.int16)
    slot_i = e_rank * 128 + e_r
    e_off = np.where(e_isA, offs_A[e_t], offs_B[e_t])
    idx16[e_core, slot_i % 16, e_off + slot_i // 16] = e_val
    idx_full = np.tile(idx16, (1, 8, 1))

    in_maps = []
    for c in range(NCORES):
        nodes_c = nodes_by_core[c]
        xT = np.zeros((IN_F, NPC), np.float32)
        xT[:, : len(nodes_c)] = x[nodes_c].T
        dv = np.zeros(NPC, np.float32)
        dv[: len(nodes_c)] = dinv[nodes_c]
        dv_t = dv.reshape(NTILES, 128).T
        in_maps.append(dict(
            xT=np.ascontiguousarray(xT),
            idx=np.ascontiguousarray(idx_full[c]),
            dnm0=np.ascontiguousarray(dv_t),
            dnm1=np.ascontiguousarray(-dv_t),
            dnm2=np.ascontiguousarray(-dv_t * dv_t),
            dnm3=np.ascontiguousarray(-2.0 * dv_t),
        ))

    meta = dict(D_A=[int(v) for v in D_A], D_B=[int(v) for v in D_B],
                offs_A=[int(v) for v in offs_A], offs_B=[int(v) for v in offs_B],
                totcols=totcols, nodes_by_core=nodes_by_core)
    return in_maps, meta


def prep_weights(W1, b1, W_rest, b_rest, gamma, beta, lin_w, lin_b):
    W1 = np.asarray(W1, np.float32)
    W_rest = np.asarray(W_rest, np.float32)
    # mid weights: [64, 9*64], layer li (1..9) block = cols [(li-1)*64, li*64)
    mid_wd = np.concatenate([W_rest[i, 0] - W_rest[i, 2] for i in range(9)], axis=1)
    mid_w1 = np.concatenate([W_rest[i, 1] for i in range(9)], axis=1)
    mid_w2 = np.concatenate([W_rest[i, 2] for i in range(9)], axis=1)
    return dict(
        l1_wd=np.ascontiguousarray(W1[0] - W1[2]),
        l1_w1=np.ascontiguousarray(W1[1]),
        l1_w2=np.ascontiguousarray(W1[2]),
        mid_wd=np.ascontiguousarray(mid_wd),
        mid_w1=np.ascontiguousarray(mid_w1),
        mid_w2=np.ascontiguousarray(mid_w2),
        ball=np.ascontiguousarray(
            np.concatenate([np.asarray(b1, np.float32)[None, :],
                            np.asarray(b_rest, np.float32)], axis=0).T),
        gall=np.ascontiguousarray(np.asarray(gamma, np.float32).T),
        beall=np.ascontiguousarray(np.asarray(beta, np.float32).T),
        linw=np.ascontiguousarray(np.asarray(lin_w, np.float32)),
        linb=np.ascontiguousarray(np.asarray(lin_b, np.float32).reshape(OUT_F, 1)),
    )


# ========================= device kernel =========================

def build_nc(meta):
    D_A, D_B = meta["D_A"], meta["D_B"]
    offs_A, offs_B = meta["offs_A"], meta["offs_B"]
    totcols = meta["totcols"]
    DT = [a + b for a, b in zip(D_A, D_B)]

    nc = bacc.Bacc("TRN2", target_bir_lowering=False, debug=False,
                   num_devices=NCORES, num_swdge_queues=4)

    xT_d = nc.dram_tensor("xT", [IN_F, NPC], FP, kind="ExternalInput")
    idx_d = nc.dram_tensor("idx", [128, totcols], I16, kind="ExternalInput")
    dnm_d = {k: nc.dram_tensor(k, [128, NTILES], FP, kind="ExternalInput")
             for k in ("dnm0", "dnm1", "dnm2", "dnm3")}
    w_d = {}
    for k, shp in (("l1_wd", [IN_F, C]), ("l1_w1", [IN_F, C]), ("l1_w2", [IN_F, C]),
                   ("mid_wd", [C, 9 * C]), ("mid_w1", [C, 9 * C]),
                   ("mid_w2", [C, 9 * C]), ("ball", [C, 10]), ("gall", [C, 10]),
                   ("beall", [C, 10]), ("linw", [C, OUT_F]), ("linb", [OUT_F, 1])):
        w_d[k] = nc.dram_tensor(k, shp, FP, kind="ExternalInput")
    y_d = nc.dram_tensor("yT", [OUT_F, NPC], FP, kind="ExternalOutput")

    groups = [list(range(NCORES))]

    with tile.TileContext(nc) as tc:
        nc.gpsimd.load_library(library_config.mlp)
        with (
            tc.tile_pool(name="dram", bufs=1, space="DRAM") as dram,
            tc.tile_pool(name="persist", bufs=1) as per,
            tc.tile_pool(name="stagp", bufs=4) as stp,
            tc.tile_pool(name="work", bufs=3) as wk,
            tc.tile_pool(name="bn", bufs=1) as bnp,
            tc.tile_pool(name="psum", bufs=2, space="PSUM") as pp,
        ):
            U = [dram.tile([NTOT, C], FP, tag=f"U{i}", name=f"U{i}") for i in range(3)]
            AGIN = [dram.tile([NPC, C], FP, tag=f"AGIN{i}", name=f"AGIN{i}") for i in range(2)]
            st_in = dram.tile([C, 2], FP, tag="st_in")
            st_out = dram.tile([C, 2], FP, tag="st_out")

            xT = per.tile([IN_F, NPC], FP)
            idxs = per.tile([128, totcols], I16)
            dnm = {k: per.tile([128, NTILES], FP, tag=k, name=f"sb_{k}") for k in dnm_d}
            hT = per.tile([C, NPC], FP)
            auxT = per.tile([C, NPC], FP)
            ubuf = per.tile([128, NTILES * C], FP)
            ubuf2 = per.tile([128, NTILES * C], FP)
            sums = per.tile([C, NTILES], FP)
            sumsq = per.tile([C, NTILES], FP)
            ident = per.tile([128, 128], FP)
            w_sb = {k: per.tile(w_d[k].shape, FP, tag=k, name=f"sb_{k}") for k in w_d}

            masks.make_identity(nc, ident[:])
            nc.sync.dma_start(xT[:], xT_d[:])
            nc.sync.dma_start(idxs[:], idx_d[:])
            for k in dnm:
                nc.sync.dma_start(dnm[k][:], dnm_d[k][:])
            for k in w_sb:
                nc.sync.dma_start(w_sb[k][:], w_d[k][:])

            def cols(t, n=128):
                return slice(t * 128, t * 128 + n)

            qcnt = [0]

            def gather_tile(t, src, stag):
                DMAX = 8
                for (off_d, Dh, coff, s0, s1) in (
                        (0, D_A[t], offs_A[t], 0, HALF),
                        (D_A[t], D_B[t], offs_B[t], HALF, NTOT)):
                    for k in range(0, Dh, DMAX):
                        dk = min(DMAX, Dh - k)
                        n = 128 * dk
                        nc.gpsimd.dma_gather(
                            stag[:, off_d + k:off_d + k + dk, :],
                            src[s0:s1, :],
                            idxs[:, coff + 8 * k:coff + 8 * (k + dk)],
                            n, n, C, queue_num=qcnt[0] % 4)
                        qcnt[0] += 1

            def gather_reduce(t, src):
                """-> S [128, C] = sum of gathered u rows for tile t."""
                stag = stp.tile([128, DT[t], C], FP, tag="stag")
                gather_tile(t, src, stag)
                S = wk.tile([128, C], FP, tag="S")
                nc.vector.reduce_sum(
                    S[:], stag[:, :, :].rearrange("p d f -> p f d"),
                    axis=mybir.AxisListType.X)
                return S

            def transpose_to_sbuf(dst_ap, src_nm_ap, pdim=128):
                """[pdim, C] node-major -> [C, pdim] SBUF via PE + ACT copy."""
                ps = pp.tile([C, 128], FP, tag="pstr")
                nc.tensor.transpose(ps[:, :pdim], src_nm_ap, ident[:pdim, :pdim])
                nc.scalar.activation(dst_ap, ps[:, :pdim], AF.Copy)

            def upload_tile(srcT_ap, t, scale_ap, dst_ubuf):
                """dst_ubuf[:, t*C:(t+1)*C] = transpose(srcT_ap)*scale.
                srcT_ap: [C, 128] feature-major SBUF."""
                ps = pp.tile([128, C], FP, tag="psup")
                nc.tensor.transpose(ps[:], srcT_ap, ident[:C, :C])
                nc.vector.tensor_scalar_mul(
                    dst_ubuf[:, t * C:(t + 1) * C], ps[:], scale_ap)

            def dma_ubuf(src_ubuf, agin):
                nc.sync.dma_start(
                    agin[:].rearrange("(t r) f -> r t f", r=128),
                    src_ubuf[:].rearrange("p (t f) -> p t f", f=C))

            def allgather(agin, uout):
                nc.gpsimd.collective_compute(
                    "AllGather", OP.bypass, replica_groups=groups,
                    ins=[agin.opt()], outs=[uout.opt()])

            def bn_stats_and_affine(li):
                """Reduce per-tile sums, AllReduce, compute a/c, return APs."""
                stpack = bnp.tile([C, 2], FP, tag="stpack")
                nc.vector.reduce_sum(stpack[:, 0:1], sums[:],
                                     axis=mybir.AxisListType.X)
                nc.vector.reduce_sum(stpack[:, 1:2], sumsq[:],
                                     axis=mybir.AxisListType.X)
                nc.sync.dma_start(st_in[:], stpack[:])
                nc.gpsimd.collective_compute(
                    "AllReduce", OP.add, replica_groups=groups,
                    ins=[st_in.opt()], outs=[st_out.opt()])
                gst = bnp.tile([C, 2], FP, tag="gst")
                nc.sync.dma_start(gst[:], st_out[:])
                mu = bnp.tile([C, 1], FP, tag="mu")
                ex2 = bnp.tile([C, 1], FP, tag="ex2")
                var = bnp.tile([C, 1], FP, tag="var")
                std = bnp.tile([C, 1], FP, tag="std")
                istd = bnp.tile([C, 1], FP, tag="istd")
                a = bnp.tile([C, 1], FP, tag="a")
                cvec = bnp.tile([C, 1], FP, tag="cvec")
                nc.vector.tensor_scalar_mul(mu[:], gst[:, 0:1], 1.0 / N)
                nc.vector.tensor_scalar_mul(ex2[:], gst[:, 1:2], 1.0 / N)
                nc.vector.tensor_mul(var[:], mu[:], mu[:])
                nc.vector.tensor_sub(var[:], ex2[:], var[:])
                nc.vector.tensor_scalar_add(var[:], var[:], EPS)
                nc.scalar.activation(std[:], var[:], AF.Sqrt)
                nc.vector.reciprocal(istd[:], std[:])
                nc.vector.tensor_mul(a[:], w_sb["gall"][:, li:li + 1], istd[:])
                nc.vector.tensor_mul(cvec[:], a[:], mu[:])
                nc.vector.tensor_sub(cvec[:], w_sb["beall"][:, li:li + 1], cvec[:])
                return a, cvec

            def relu_stats(ps_ap, t, li):
                """ps (psum [C,128]) -> relu+bias into hT[:, tile t] (in place
                as g), accumulate stats."""
                rt = 128 if t < NTILES - 1 else LAST_REAL
                if rt < 128:
                    nc.vector.memset(hT[:, cols(t)], 0.0)
                nc.scalar.activation(
                    hT[:, cols(t, rt)], ps_ap[:, 0:rt], AF.Relu,
                    bias=w_sb["ball"][:, li:li + 1], accum_out=sums[:, t:t + 1])
                sq = wk.tile([C, 128], FP, tag="sqs")
                nc.scalar.activation(
                    sq[:, 0:rt], hT[:, cols(t, rt)], AF.Square,
                    accum_out=sumsq[:, t:t + 1])

            def bn_tail(li, last):
                """affine h = a*g + c, then upload u1 = dinv*h and AllGather."""
                a, cvec = bn_stats_and_affine(li)
                for t in range(NTILES):
                    nc.vector.tensor_scalar(
                        hT[:, cols(t)], hT[:, cols(t)], a[:], cvec[:],
                        op0=OP.mult, op1=OP.add)
                    if not last:
                        upload_tile(hT[:, cols(t)], t, dnm["dnm0"][:, t:t + 1],
                                    ubuf2)
                if not last:
                    dma_ubuf(ubuf2, AGIN[1])
                    allgather(AGIN[1], U[0])

            if STAGE < 3:
                nc.vector.memset(hT[:], 0.0)

            # ================= layer 1 =================
            for t in range(NTILES):
                ps1 = pp.tile([C, 128], FP, tag="pmm")
                nc.tensor.matmul(ps1[:], w_sb["l1_w1"][:], xT[:, cols(t)],
                                 start=True, stop=True)
                z1t = wk.tile([C, 128], FP, tag="z1t")
                nc.scalar.activation(z1t[:], ps1[:], AF.Copy)
                upload_tile(z1t[:], t, dnm["dnm0"][:, t:t + 1], ubuf)

                ps2 = pp.tile([C, 128], FP, tag="pmm")
                nc.tensor.matmul(ps2[:], w_sb["l1_w2"][:], xT[:, cols(t)],
                                 start=True, stop=True)
                z2t = wk.tile([C, 128], FP, tag="z1t")
                nc.scalar.activation(z2t[:], ps2[:], AF.Copy)
                upload_tile(z2t[:], t, dnm["dnm0"][:, t:t + 1], ubuf2)
            dma_ubuf(ubuf, AGIN[0])
            dma_ubuf(ubuf2, AGIN[1])
            allgather(AGIN[0], U[0])   # U0 = ua = dinv*z1 (all nodes)
            allgather(AGIN[1], U[1])   # U1 = ub = dinv*z2

            # hop A (La = Lhat z1) and hop B (u_c upload)
            for t in range(NTILES if STAGE >= 2 else 0):
                Sa = gather_reduce(t, U[0])
                la = wk.tile([128, C], FP, tag="lanm")
                nc.vector.tensor_scalar_mul(la[:], Sa[:], dnm["dnm1"][:, t:t + 1])
                transpose_to_sbuf(auxT[:, cols(t)], la[:])
                Sb = gather_reduce(t, U[1])
                nc.vector.tensor_scalar_mul(
                    ubuf[:, t * C:(t + 1) * C], Sb[:], dnm["dnm2"][:, t:t + 1])
            if STAGE >= 3:
                dma_ubuf(ubuf, AGIN[0])
                allgather(AGIN[0], U[2])   # U2 = u_c = dinv*Lhat(z2)

            # hop C + assemble layer-1 output
            for t in range(NTILES if STAGE >= 3 else 0):
                Sc = gather_reduce(t, U[2])
                l2 = wk.tile([128, C], FP, tag="lanm")
                nc.vector.tensor_scalar_mul(l2[:], Sc[:], dnm["dnm3"][:, t:t + 1])
                ps = pp.tile([C, 128], FP, tag="pmm")
                nc.tensor.matmul(ps[:], w_sb["l1_wd"][:], xT[:, cols(t)],
                                 start=True, stop=True)
                l2t = wk.tile([C, 128], FP, tag="t2t")
                transpose_to_sbuf(l2t[:], l2[:])
                t1 = wk.tile([C, 128], FP, tag="t1")
                nc.vector.tensor_add(t1[:], ps[:], l2t[:])
                nc.vector.tensor_add(t1[:], t1[:], auxT[:, cols(t)])
                relu_stats(t1, t, 0)
            if STAGE >= 3:
                bn_tail(0, last=(STAGE < 4 or NMID == 0))

            # ================= layers 2..10 =================
            for li in range(1, (1 + NMID) if STAGE >= 4 else 1):
                wsl = slice((li - 1) * C, li * C)
                # phase 1: S1 from U0 -> u2 upload + Tx1^T
                for t in range(NTILES):
                    S1 = gather_reduce(t, U[0])
                    nc.vector.tensor_scalar_mul(
                        ubuf[:, t * C:(t + 1) * C], S1[:],
                        dnm["dnm2"][:, t:t + 1])
                    tx = wk.tile([128, C], FP, tag="lanm")
                    nc.vector.tensor_scalar_mul(tx[:], S1[:],
                                                dnm["dnm1"][:, t:t + 1])
                    transpose_to_sbuf(auxT[:, cols(t)], tx[:])
                dma_ubuf(ubuf, AGIN[0])
                allgather(AGIN[0], U[1])

                # phase 2: S2 from U1 -> out = h@Wd + Tx1@W1 + (2 L Tx1)@W2
                for t in range(NTILES):
                    S2 = gather_reduce(t, U[1])
                    l2 = wk.tile([128, C], FP, tag="lanm")
                    nc.vector.tensor_scalar_mul(l2[:], S2[:],
                                                dnm["dnm3"][:, t:t + 1])
                    t2t = wk.tile([C, 128], FP, tag="t2t")
                    transpose_to_sbuf(t2t[:], l2[:])
                    ps = pp.tile([C, 128], FP, tag="pmm")
                    nc.tensor.matmul(ps[:], w_sb["mid_wd"][:, wsl],
                                     hT[:, cols(t)], start=True, stop=False)
                    nc.tensor.matmul(ps[:], w_sb["mid_w1"][:, wsl],
                                     auxT[:, cols(t)], start=False, stop=False)
                    nc.tensor.matmul(ps[:], w_sb["mid_w2"][:, wsl],
                                     t2t[:], start=False, stop=True)
                    relu_stats(ps, t, li)
                bn_tail(li, last=(li == NMID))

            # ================= final linear =================
            for t in range(NTILES):
                ps = pp.tile([OUT_F, 128], FP, tag="pmm")
                nc.tensor.matmul(ps[:], w_sb["linw"][:], hT[:, cols(t)],
                                 start=True, stop=True)
                yt = wk.tile([OUT_F, 128], FP, tag="yt")
                nc.vector.tensor_scalar_add(yt[:], ps[:], w_sb["linb"][:])
                nc.sync.dma_start(y_d[:, cols(t)], yt[:])

    nc.compile()
    return nc


# ========================= public entry =========================

_CACHE = {}


def _get_compiled(meta):
    key = (tuple(meta["D_A"]), tuple(meta["D_B"]), meta["totcols"], STAGE, NMID)
    if key not in _CACHE:
        _CACHE[key] = build_nc(meta)
    return _CACHE[key]


def kernel(x, edge_index, W1, b1, W_rest, b_rest, gamma, beta, lin_w, lin_b):
    in_maps, meta = preprocess(x, edge_index)
    wm = prep_weights(W1, b1, W_rest, b_rest, gamma, beta, lin_w, lin_b)
    for m in in_maps:
        m.update(wm)
    nc = _get_compiled(meta)
    res = run_bass_kernel_spmd(nc, in_maps, core_ids=list(range(NCORES)))
    y = np.empty((N, OUT_F), np.float32)
    for c in range(NCORES):
        nodes_c = meta["nodes_by_core"][c]
        y[nodes_c] = res.results[c]["yT"][:, :len(nodes_c)].T
    return y

